# revision 13
# baseline (speedup 1.0000x reference)
"""AttentionPairBias Trainium2 kernel — 8-core SPMD, head-sharded (2 heads/core).

Core m owns output rows [128m, 128m+128) == heads {2m, 2m+1}.  Host side does
layout-only prep (slicing, transposes, dtype casts); all reference FLOPs run
on device.  See layout_check.py for the numpy mock this was validated against.

Device dataflow per core:
 - z phase: z arrives host-transposed as [s1-pair, (parity,cz)=128, s2=1024]
   bf16.  One block-diagonal [128,36] lhsT computes, per site, the 16-channel
   u-projection (u = pnorm_w*bias_w) + sum(z); a second matmul over ACT-squared
   z fills sum(z^2).  Results bounce through DRAM scratch laid out [s1][it][s2]
   so the later reload lands directly as [x'-partition, y'-free] bias tiles.
   LN is algebraically folded: bias = r*(P - m*U) + (C + bias_b).
 - a1 = sigmoid((s_n@pb_wT + pb_b)*a_n + s_n@pn_wT); q/kvg projections with
   host-pre-transposed bf16 weights (kvg columns host-permuted to (v,j,ch)).
 - attention rows indexed in sigma order x' = 64*j + rl (s2 = 16*rl + j) so
   every head-split gather is a 64x64 PE transpose or identity-matmul
   partition shift.  Softmax over the free axis without max-subtraction
   (scores ~ N(0, 0.3)); denominators from exp accum_out, folded into V rows.
 - o computed transposed [ch, y'], gated by gT, retiled to GO^T k-tiles via
   identity matmuls, then attn/out projections and final sigmoid gating.
"""
import os
import numpy as np
import ml_dtypes

BF16 = ml_dtypes.bfloat16
EPS = 1e-5
S = 1024
CA = 1024
CS = 512
CZ = 64
C = 64
NCORES = 8

_cache = {}


def _build_program(debug=False):
    import concourse.bass as bass
    import concourse.tile as tile
    from concourse import mybir, bacc
    from contextlib import ExitStack

    fp32 = mybir.dt.float32
    bf16 = mybir.dt.bfloat16
    AF = mybir.ActivationFunctionType
    OP = mybir.AluOpType
    AX = mybir.AxisListType

    nc = bacc.Bacc("TRN2", target_bir_lowering=False, debug=False)

    P_ = nc.declare_dram_parameter
    a_loc = P_("a_loc", [128, CA], fp32, isOutput=False)
    s_loc = P_("s_loc", [128, CS], fp32, isOutput=False)
    sT_loc = P_("sT_loc", [CS, 128], bf16, isOutput=False)
    z_t = P_("z_t", [64, 128, S], bf16, isOutput=False)
    pb_wT = P_("pb_wT", [CS, CA], bf16, isOutput=False)
    pn_wT = P_("pn_wT", [CS, CA], bf16, isOutput=False)
    q_wT = P_("q_wT", [CA, CA], bf16, isOutput=False)
    kvg_wT = P_("kvg_wT", [CA, 3 * CA], bf16, isOutput=False)
    attn_wT = P_("attn_wT", [CA, CA], bf16, isOutput=False)
    out_wT = P_("out_wT", [CS, CA], bf16, isOutput=False)
    bias_wT2 = P_("bias_wT2", [128, 16], fp32, isOutput=False)
    pnw2 = P_("pnw2", [128, 1], fp32, isOutput=False)
    pnormb_col = P_("pnormb_col", [64, 1], fp32, isOutput=False)
    biasb_col = P_("biasb_col", [16, 1], fp32, isOutput=False)
    snw4 = P_("snw4", [128, 4], fp32, isOutput=False)
    pb_b_r = P_("pb_b_r", [1, CA], fp32, isOutput=False)
    qb_r = P_("qb_r", [1, CA], fp32, isOutput=False)
    outb_r = P_("outb_r", [1, CA], fp32, isOutput=False)
    id128 = P_("id128", [128, 128], bf16, isOutput=False)
    out_p = P_("out", [128, CA], fp32, isOutput=True)

    dbg = {}
    if debug:
        for nm, shp in [("d_a1", [128, CA]), ("d_q", [128, CA]),
                        ("d_kvg", [128, 3 * CA]), ("d_stats", [36, 512]),
                        ("d_bias0", [128, S]), ("d_E0", [128, S]),
                        ("d_oT0", [64, S]), ("d_goT", [128, 8 * 128]),
                        ("d_a2", [128, CA]), ("d_snT", [128, 512]),
                        ("d_KT0", [64, S]), ("d_QT0", [64, S]),
                        ("d_V0", [128, 8 * 64])]:
            dbg[nm] = P_(nm, shp, fp32, isOutput=True)

    with ExitStack() as ctx:
        tc = ctx.enter_context(tile.TileContext(nc))
        const = ctx.enter_context(tc.tile_pool(name="const", bufs=1))
        dramp = ctx.enter_context(tc.tile_pool(name="dramp", bufs=1, space="DRAM"))
        wpool = ctx.enter_context(tc.tile_pool(name="wpool", bufs=3))
        zpool = ctx.enter_context(tc.tile_pool(name="zpool", bufs=3))
        spool = ctx.enter_context(tc.tile_pool(name="spool", bufs=2))
        apool = ctx.enter_context(tc.tile_pool(name="apool", bufs=1))
        hpool = ctx.enter_context(tc.tile_pool(name="hpool", bufs=2))
        epool = ctx.enter_context(tc.tile_pool(name="epool", bufs=3))
        pssc = ctx.enter_context(tc.tile_pool(name="pssc", bufs=4, space="PSUM"))
        psaux = ctx.enter_context(tc.tile_pool(name="psaux", bufs=2, space="PSUM"))
        psav = ctx.enter_context(tc.tile_pool(name="psav", bufs=2, space="PSUM"))

        biasP = dramp.tile([128, 16, S], bf16, tag="biasP")   # [s1][it][s2]
        momS = dramp.tile([128, 2, S], fp32, tag="momS")      # [s1][M1|M2][s2]

        # ---------------- constants ----------------
        idt = const.tile([128, 128], bf16, tag="idt")
        nc.sync.dma_start(idt[:], id128[:])
        bwT2 = const.tile([128, 16], fp32, tag="bwT2")
        nc.sync.dma_start(bwT2[:], bias_wT2[:])
        bwT2b = const.tile([128, 16], bf16, tag="bwT2b")
        nc.vector.tensor_copy(bwT2b[:], bwT2[:])
        pnw2_t = const.tile([128, 1], fp32, tag="pnw2t")
        nc.sync.dma_start(pnw2_t[:], pnw2[:])
        uT2 = const.tile([128, 16], bf16, tag="uT2")
        nc.vector.tensor_scalar_mul(uT2[:], bwT2[:], pnw2_t[:])

        W36 = const.tile([128, 36], bf16, tag="W36")
        nc.vector.memset(W36[:], 0.0)
        nc.vector.tensor_copy(W36[0:64, 0:16], uT2[0:64, :])
        nc.vector.tensor_copy(W36[64:128, 18:34], uT2[64:128, :])
        nc.vector.memset(W36[0:64, 16:17], 1.0)
        nc.vector.memset(W36[64:128, 34:35], 1.0)
        W36q = const.tile([128, 36], bf16, tag="W36q")
        nc.vector.memset(W36q[:], 0.0)
        nc.vector.memset(W36q[0:64, 17:18], 1.0)
        nc.vector.memset(W36q[64:128, 35:36], 1.0)

        ones_col = const.tile([64, 1], bf16, tag="ones_col")
        nc.vector.memset(ones_col[:], 1.0)
        pnb_col = const.tile([64, 1], bf16, tag="pnb_col")
        nc.gpsimd.dma_start(pnb_col[:], pnormb_col[:])
        bb_col = const.tile([16, 1], fp32, tag="bb_col")
        nc.sync.dma_start(bb_col[:], biasb_col[:])

        ps_u = psaux.tile([128, 128], fp32, tag="aux")
        nc.tensor.matmul(ps_u[0:16, 0:1], uT2[0:64, :], ones_col[:], start=True, stop=True)
        ps_c = psaux.tile([128, 128], fp32, tag="aux")
        nc.tensor.matmul(ps_c[0:16, 0:1], bwT2b[0:64, :], pnb_col[:], start=True, stop=True)
        UCcol = const.tile([16, 2], bf16, tag="UCcol")
        nc.vector.tensor_copy(UCcol[:, 0:1], ps_u[0:16, 0:1])
        CCp = const.tile([16, 1], fp32, tag="CCp")
        nc.vector.tensor_copy(CCp[:], ps_c[0:16, 0:1])
        CCc = const.tile([16, 1], fp32, tag="CCc")
        nc.vector.tensor_add(CCc[:], CCp[:], bb_col[:])
        nc.vector.tensor_copy(UCcol[:, 1:2], CCc[:])
        ps_t = psaux.tile([128, 128], bf16, tag="aux")
        nc.tensor.transpose(ps_t[0:1, 0:16], UCcol[:, 0:1], idt[0:16, 0:16])
        ps_t2 = psaux.tile([128, 128], bf16, tag="aux")
        nc.tensor.transpose(ps_t2[0:1, 0:16], UCcol[:, 1:2], idt[0:16, 0:16])
        U_row = const.tile([1, 16], fp32, tag="U_row")
        nc.vector.tensor_copy(U_row[:], ps_t[0:1, 0:16])
        CC_row = const.tile([1, 16], fp32, tag="CC_row")
        nc.vector.tensor_copy(CC_row[:], ps_t2[0:1, 0:16])
        U_b = const.tile([128, 16], fp32, tag="U_b")
        nc.gpsimd.partition_broadcast(U_b[:], U_row[0:1, :])
        CC_b = const.tile([128, 16], fp32, tag="CC_b")
        nc.gpsimd.partition_broadcast(CC_b[:], CC_row[0:1, :])

        row_t = const.tile([1, 3 * CA], fp32, tag="row_t")
        nc.sync.dma_start(row_t[0:1, 0:CA], pb_b_r[:])
        nc.sync.dma_start(row_t[0:1, CA:2 * CA], qb_r[:])
        nc.sync.dma_start(row_t[0:1, 2 * CA:3 * CA], outb_r[:])
        pbb_b = const.tile([128, CA], fp32, tag="pbb_b")
        nc.gpsimd.partition_broadcast(pbb_b[:], row_t[0:1, 0:CA])
        qb_b = const.tile([128, CA], fp32, tag="qb_b")
        nc.gpsimd.partition_broadcast(qb_b[:], row_t[0:1, CA:2 * CA])
        nc.vector.tensor_scalar_mul(qb_b[:], qb_b[:], 1.0 / C)
        outb_b = const.tile([128, CA], fp32, tag="outb_b")
        nc.gpsimd.partition_broadcast(outb_b[:], row_t[0:1, 2 * CA:3 * CA])
        snw_t = const.tile([128, 4], fp32, tag="snw_t")
        nc.sync.dma_start(snw_t[:], snw4[:])
        eps_col = const.tile([128, 1], fp32, tag="eps_col")
        nc.vector.memset(eps_col[:], EPS)

        # ---------------- z phase ----------------
        for i in range(64):
            zt = zpool.tile([128, S], bf16, tag="zt")
            nc.sync.dma_start(zt[:], z_t[i])
            zsq = zpool.tile([128, S], bf16, tag="zsq")
            nc.scalar.square(zsq[:], zt[:])
            for cch in range(2):
                sl = slice(512 * cch, 512 * (cch + 1))
                ps_st = pssc.tile([128, 512], fp32, tag="big")
                nc.tensor.matmul(ps_st[0:36, :], W36[:], zt[:, sl], start=True, stop=False)
                nc.tensor.matmul(ps_st[0:36, :], W36q[:], zsq[:, sl], start=False, stop=True)
                st_bf = spool.tile([36, 512], bf16, tag="stbf")
                nc.scalar.activation(st_bf[:], ps_st[0:36, :], AF.Copy)
                st_f = spool.tile([36, 512], fp32, tag="stf")
                nc.vector.tensor_copy(st_f[:], ps_st[0:36, :])
                nc.sync.dma_start(biasP[2 * i, :, sl], st_bf[0:16, :])
                nc.sync.dma_start(biasP[2 * i + 1, :, sl], st_bf[18:34, :])
                nc.sync.dma_start(momS[2 * i, :, sl], st_f[16:18, :])
                nc.sync.dma_start(momS[2 * i + 1, :, sl], st_f[34:36, :])
                if debug and i == 0 and cch == 0:
                    nc.sync.dma_start(dbg["d_stats"][:], st_f[:])

        # ---------------- LN(a), LN(s), a1 ----------------
        a_t = apool.tile([128, CA], fp32, tag="a_t")
        nc.sync.dma_start(a_t[:], a_loc[:])
        s_t = apool.tile([128, CS], fp32, tag="s_t")
        nc.sync.dma_start(s_t[:], s_loc[:])

        def ln_stats(x, n, tg):
            xsq = spool.tile([128, n], bf16, tag="lnsq")
            ssq = spool.tile([128, 1], fp32, tag=tg + "ss")
            nc.scalar.activation(xsq[:], x[:], AF.Square, accum_out=ssq[:])
            mt = spool.tile([128, 1], fp32, tag=tg + "m")
            nc.vector.reduce_sum(mt[:], x[:], axis=AX.X)
            nc.vector.tensor_scalar_mul(mt[:], mt[:], 1.0 / n)
            mm = spool.tile([128, 1], fp32, tag=tg + "mm")
            nc.vector.tensor_mul(mm[:], mt[:], mt[:])
            vt = spool.tile([128, 1], fp32, tag=tg + "v")
            nc.vector.tensor_scalar(vt[:], ssq[:], 1.0 / n, None, OP.mult)
            nc.vector.tensor_sub(vt[:], vt[:], mm[:])
            sq = spool.tile([128, 1], fp32, tag=tg + "sq")
            nc.scalar.activation(sq[:], vt[:], AF.Sqrt, bias=eps_col[:])
            rt = spool.tile([128, 1], fp32, tag=tg + "r")
            nc.vector.reciprocal(rt[:], sq[:])
            return mt, rt

        am, ar = ln_stats(a_t, CA, "aln")
        a_n = apool.tile([128, CA], bf16, tag="a_n")
        nc.vector.tensor_scalar(a_n[:], a_t[:], am[:], ar[:], OP.subtract, OP.mult)
        sm, sr = ln_stats(s_t, CS, "sln")
        s_n = apool.tile([128, CS], bf16, tag="s_n")
        nc.vector.tensor_scalar(s_n[:], s_t[:], sm[:], sr[:], OP.subtract, OP.mult)

        s_nT = apool.tile([128, 512], bf16, tag="s_nT")
        for k in range(4):
            ps = psaux.tile([128, 128], bf16, tag="aux")
            nc.tensor.transpose(ps[:], s_n[:, 128 * k:128 * (k + 1)], idt[:])
            nc.vector.tensor_scalar_mul(s_nT[:, 128 * k:128 * (k + 1)], ps[:], snw_t[:, k:k + 1])
        if debug:
            dsn = spool.tile([128, 512], fp32, tag="dbgcp")
            nc.vector.tensor_copy(dsn[:], s_nT[:])
            nc.sync.dma_start(dbg["d_snT"][:], dsn[:])

        ps_a = [pssc.tile([128, 512], fp32, tag="big", name=f"ps_a{i_}") for i_ in range(4)]
        for k in range(4):
            wb = wpool.tile([128, CA], bf16, tag="wpb")
            nc.sync.dma_start(wb[:], pb_wT[128 * k:128 * (k + 1), :])
            wn = wpool.tile([128, CA], bf16, tag="wpn")
            nc.sync.dma_start(wn[:], pn_wT[128 * k:128 * (k + 1), :])
            lt = s_nT[:, 128 * k:128 * (k + 1)]
            nc.tensor.matmul(ps_a[0][:], lt, wb[:, 0:512], start=(k == 0), stop=(k == 3))
            nc.tensor.matmul(ps_a[1][:], lt, wb[:, 512:1024], start=(k == 0), stop=(k == 3))
            nc.tensor.matmul(ps_a[2][:], lt, wn[:, 0:512], start=(k == 0), stop=(k == 3))
            nc.tensor.matmul(ps_a[3][:], lt, wn[:, 512:1024], start=(k == 0), stop=(k == 3))
        a1 = apool.tile([128, CA], bf16, tag="a1")
        for n in range(2):
            sl = slice(512 * n, 512 * (n + 1))
            t0 = spool.tile([128, 512], fp32, tag="a1t")
            nc.vector.tensor_add(t0[:], ps_a[n][:], pbb_b[:, sl])
            nc.vector.tensor_mul(t0[:], t0[:], a_n[:, sl])
            nc.vector.tensor_add(t0[:], t0[:], ps_a[2 + n][:])
            nc.scalar.activation(a1[:, sl], t0[:], AF.Sigmoid)
        if debug:
            dd = spool.tile([128, CA], fp32, tag="dbgcp")
            nc.vector.tensor_copy(dd[:], a1[:])
            nc.sync.dma_start(dbg["d_a1"][:], dd[:])

        a1T = apool.tile([128, 8 * 128], bf16, tag="a1T")
        for k in range(8):
            ps = psaux.tile([128, 128], bf16, tag="aux")
            nc.tensor.transpose(ps[:], a1[:, 128 * k:128 * (k + 1)], idt[:])
            nc.vector.tensor_copy(a1T[:, 128 * k:128 * (k + 1)], ps[:])

        q_sb = apool.tile([128, CA], bf16, tag="q_sb")
        for n in range(2):
            ps = pssc.tile([128, 512], fp32, tag="big")
            for k in range(8):
                wq = wpool.tile([128, 512], bf16, tag="wq")
                nc.sync.dma_start(wq[:], q_wT[128 * k:128 * (k + 1), 512 * n:512 * (n + 1)])
                nc.tensor.matmul(ps[:], a1T[:, 128 * k:128 * (k + 1)], wq[:], start=(k == 0), stop=(k == 7))
            nc.vector.scalar_tensor_tensor(q_sb[:, 512 * n:512 * (n + 1)], ps[:], 1.0 / C,
                                           qb_b[:, 512 * n:512 * (n + 1)], OP.mult, OP.add)
        kvg_sb = apool.tile([128, 3 * CA], bf16, tag="kvg_sb")
        for n in range(6):
            ps = pssc.tile([128, 512], fp32, tag="big")
            for k in range(8):
                wk = wpool.tile([128, 512], bf16, tag="wkvg")
                nc.sync.dma_start(wk[:], kvg_wT[128 * k:128 * (k + 1), 512 * n:512 * (n + 1)])
                nc.tensor.matmul(ps[:], a1T[:, 128 * k:128 * (k + 1)], wk[:], start=(k == 0), stop=(k == 7))
            nc.vector.tensor_copy(kvg_sb[:, 512 * n:512 * (n + 1)], ps[:])
        if debug:
            dq = spool.tile([128, CA], fp32, tag="dbgcp")
            nc.vector.tensor_copy(dq[:], q_sb[:])
            nc.sync.dma_start(dbg["d_q"][:], dq[:])
            for n in range(3):
                dk = spool.tile([128, CA], fp32, tag="dbgcp")
                nc.vector.tensor_copy(dk[:], kvg_sb[:, CA * n:CA * (n + 1)])
                nc.sync.dma_start(dbg["d_kvg"][:, CA * n:CA * (n + 1)], dk[:])

        gsig = apool.tile([128, CA], bf16, tag="gsig")
        nc.scalar.activation(gsig[:], kvg_sb[:, 2 * CA:3 * CA], AF.Sigmoid)

        # ---------------- attention ----------------
        go_T = apool.tile([128, 8 * 128], bf16, tag="go_T")
        biasP_r = biasP.rearrange("a b (c d) -> a b c d", d=64)   # [s1][it][jk][rq]
        momS_r = momS.rearrange("a b (c d) -> a b c d", d=64)
        for l in range(2):
            sl_h = slice(64 * l, 64 * l + 64)
            eye = idt[sl_h, sl_h]
            KT = hpool.tile([64, S], bf16, tag="KT")
            QT = hpool.tile([64, S], bf16, tag="QT")
            gT = hpool.tile([64, S], bf16, tag="gT")
            for grp in range(2):
                psK = psav.tile([64, 512], bf16, tag="hav")
                psQ = psav.tile([64, 512], bf16, tag="hav")
                psG = psav.tile([64, 512], bf16, tag="hav")
                for jj in range(8):
                    j = 8 * grp + jj
                    fs = slice(64 * jj, 64 * (jj + 1))
                    nc.tensor.transpose(psK[:, fs], kvg_sb[sl_h, 64 * j:64 * j + 64], eye)
                    nc.tensor.transpose(psQ[:, fs], q_sb[sl_h, 64 * j:64 * j + 64], eye)
                    nc.tensor.transpose(psG[:, fs], gsig[sl_h, 64 * j:64 * j + 64], eye)
                gs = slice(512 * grp, 512 * (grp + 1))
                nc.vector.tensor_copy(KT[:, gs], psK[:])
                nc.vector.tensor_copy(QT[:, gs], psQ[:])
                nc.scalar.activation(gT[:, gs], psG[:], AF.Copy)
            if debug and l == 0:
                dKT = spool.tile([64, S], fp32, tag="dbgh")
                nc.vector.tensor_copy(dKT[:], KT[:])
                nc.sync.dma_start(dbg["d_KT0"][:], dKT[:])
                dQT = spool.tile([64, S], fp32, tag="dbgh")
                nc.vector.tensor_copy(dQT[:], QT[:])
                nc.sync.dma_start(dbg["d_QT0"][:], dQT[:])

            Vt = hpool.tile([128, 8 * 64], bf16, tag="Vt")
            for t in range(8):
                psV = psaux.tile([128, 128], fp32, tag="aux")
                for jj in range(2):
                    j = 2 * t + jj
                    src = kvg_sb[sl_h, CA + 64 * j:CA + 64 * j + 64]
                    nc.tensor.matmul(psV[64 * jj:64 * (jj + 1), 0:64], eye, src, start=True, stop=True)
                nc.vector.tensor_copy(Vt[:, 64 * t:64 * (t + 1)], psV[:, 0:64])
            if debug and l == 0:
                dV = spool.tile([128, 8 * 64], fp32, tag="dbgh")
                nc.vector.tensor_copy(dV[:], Vt[:])
                nc.sync.dma_start(dbg["d_V0"][:], dV[:])

            av0 = psav.tile([64, 512], fp32, tag="hav")
            av1 = psav.tile([64, 512], fp32, tag="hav")
            for t in range(8):
                ps_s0 = pssc.tile([128, 512], fp32, tag="big")
                ps_s1 = pssc.tile([128, 512], fp32, tag="big")
                nc.tensor.matmul(ps_s0[:], KT[:, 128 * t:128 * (t + 1)], QT[:, 0:512], start=True, stop=True)
                nc.tensor.matmul(ps_s1[:], KT[:, 128 * t:128 * (t + 1)], QT[:, 512:1024], start=True, stop=True)

                Pt = epool.tile([128, 16, 64], bf16, tag="Pt")
                nc.sync.dma_start(Pt[0:64, :, :], biasP_r[sl_h, :, 2 * t, :])
                nc.sync.dma_start(Pt[64:128, :, :], biasP_r[sl_h, :, 2 * t + 1, :])
                Mt = epool.tile([128, 2, 64], fp32, tag="Mt")
                nc.sync.dma_start(Mt[0:64, :, :], momS_r[sl_h, :, 2 * t, :])
                nc.sync.dma_start(Mt[64:128, :, :], momS_r[sl_h, :, 2 * t + 1, :])
                mt = epool.tile([128, 64], fp32, tag="mt")
                nc.vector.tensor_scalar_mul(mt[:], Mt[:, 0, :], 1.0 / CZ)
                vt = epool.tile([128, 64], fp32, tag="vt")
                nc.vector.tensor_mul(vt[:], mt[:], mt[:])
                nc.vector.scalar_tensor_tensor(vt[:], Mt[:, 1, :], 1.0 / CZ, vt[:], OP.mult, OP.subtract)
                rt = epool.tile([128, 64], fp32, tag="rt")
                nc.scalar.activation(rt[:], vt[:], AF.Sqrt, bias=eps_col[:])
                nc.vector.reciprocal(rt[:], rt[:])
                rmt = epool.tile([128, 64], fp32, tag="rmt")
                nc.vector.tensor_mul(rmt[:], rt[:], mt[:])

                bt = epool.tile([128, 16, 64], fp32, tag="bt")
                r3 = rt[:].rearrange("p (o d) -> p o d", o=1).to_broadcast((128, 16, 64))
                rm3 = rmt[:].rearrange("p (o d) -> p o d", o=1).to_broadcast((128, 16, 64))
                U3 = U_b[:].rearrange("p (i o) -> p i o", o=1).to_broadcast((128, 16, 64))
                CC3 = CC_b[:].rearrange("p (i o) -> p i o", o=1).to_broadcast((128, 16, 64))
                nc.vector.tensor_tensor(bt[:], Pt[:], r3, OP.mult)
                t2 = epool.tile([128, 16, 64], bf16, tag="t2")
                nc.vector.tensor_tensor(t2[:], U3, rm3, OP.mult)
                nc.vector.tensor_sub(bt[:], bt[:], t2[:])
                nc.vector.tensor_tensor(bt[:], bt[:], CC3, OP.add)
                if debug and l == 0 and t == 0:
                    nc.sync.dma_start(dbg["d_bias0"][:], bt[:].rearrange("p i d -> p (i d)"))

                bt2 = bt[:].rearrange("p i d -> p (i d)")
                Et = epool.tile([128, S], bf16, tag="Et")
                d0 = epool.tile([128, 1], fp32, tag="d0")
                d1 = epool.tile([128, 1], fp32, tag="d1")
                sc0 = epool.tile([128, 512], fp32, tag="sc")
                nc.vector.tensor_add(sc0[:], ps_s0[:], bt2[:, 0:512])
                nc.scalar.activation(Et[:, 0:512], sc0[:], AF.Exp, accum_out=d0[:])
                sc1 = epool.tile([128, 512], fp32, tag="sc")
                nc.vector.tensor_add(sc1[:], ps_s1[:], bt2[:, 512:1024])
                nc.scalar.activation(Et[:, 512:1024], sc1[:], AF.Exp, accum_out=d1[:])
                nc.vector.tensor_add(d0[:], d0[:], d1[:])
                nc.vector.reciprocal(d0[:], d0[:])
                Vp = epool.tile([128, 64], bf16, tag="Vp")
                nc.vector.tensor_scalar_mul(Vp[:], Vt[:, 64 * t:64 * (t + 1)], d0[:])
                nc.tensor.matmul(av0[:], Vp[:], Et[:, 0:512], start=(t == 0), stop=(t == 7))
                nc.tensor.matmul(av1[:], Vp[:], Et[:, 512:1024], start=(t == 0), stop=(t == 7))
                if debug and l == 0 and t == 0:
                    de = spool.tile([128, S], fp32, tag="dbgh")
                    nc.vector.tensor_copy(de[:], Et[:])
                    nc.sync.dma_start(dbg["d_E0"][:], de[:])

            goT = hpool.tile([64, S], bf16, tag="goT")
            nc.vector.tensor_tensor(goT[:, 0:512], av0[:], gT[:, 0:512], OP.mult)
            nc.vector.tensor_tensor(goT[:, 512:1024], av1[:], gT[:, 512:1024], OP.mult)
            if debug and l == 0:
                do1 = spool.tile([64, S], fp32, tag="dbgh")
                nc.vector.tensor_copy(do1[:, 0:512], av0[:])
                nc.vector.tensor_copy(do1[:, 512:1024], av1[:])
                nc.sync.dma_start(dbg["d_oT0"][:], do1[:])

            for kk in range(8):
                psg = psaux.tile([128, 128], fp32, tag="aux")
                for jj in range(2):
                    t16 = 2 * kk + jj
                    nc.tensor.matmul(psg[64 * jj:64 * (jj + 1), 64 * l:64 * l + 64],
                                     idt[0:64, 0:64], goT[:, 64 * t16:64 * t16 + 64],
                                     start=True, stop=True)
                nc.vector.tensor_copy(go_T[:, 128 * kk + 64 * l:128 * kk + 64 * l + 64],
                                      psg[:, 64 * l:64 * l + 64])
        if debug:
            dgo = spool.tile([128, 8 * 128], fp32, tag="dbgh")
            nc.vector.tensor_copy(dgo[:], go_T[:])
            nc.sync.dma_start(dbg["d_goT"][:], dgo[:])

        # ---------------- attn + out projections ----------------
        ps_a20 = pssc.tile([128, 512], fp32, tag="big")
        ps_a21 = pssc.tile([128, 512], fp32, tag="big")
        for k in range(8):
            wa = wpool.tile([128, CA], bf16, tag="wattn")
            nc.sync.dma_start(wa[:], attn_wT[128 * k:128 * (k + 1), :])
            nc.tensor.matmul(ps_a20[:], go_T[:, 128 * k:128 * (k + 1)], wa[:, 0:512], start=(k == 0), stop=(k == 7))
            nc.tensor.matmul(ps_a21[:], go_T[:, 128 * k:128 * (k + 1)], wa[:, 512:1024], start=(k == 0), stop=(k == 7))
        if debug:
            da2 = spool.tile([128, CA], fp32, tag="dbgcp")
            nc.vector.tensor_copy(da2[:, 0:512], ps_a20[:])
            nc.vector.tensor_copy(da2[:, 512:1024], ps_a21[:])
            nc.sync.dma_start(dbg["d_a2"][:], da2[:])

        sT_t = apool.tile([128, 512], bf16, tag="sT_t")
        nc.sync.dma_start(sT_t[:].rearrange("b (a c) -> b a c", a=4),
                          sT_loc.rearrange("(a b) c -> b a c", b=128))
        ps_o0 = pssc.tile([128, 512], fp32, tag="big")
        ps_o1 = pssc.tile([128, 512], fp32, tag="big")
        for k in range(4):
            wo = wpool.tile([128, CA], bf16, tag="wout")
            nc.sync.dma_start(wo[:], out_wT[128 * k:128 * (k + 1), :])
            nc.tensor.matmul(ps_o0[:], sT_t[:, 128 * k:128 * (k + 1)], wo[:, 0:512], start=(k == 0), stop=(k == 3))
            nc.tensor.matmul(ps_o1[:], sT_t[:, 128 * k:128 * (k + 1)], wo[:, 512:1024], start=(k == 0), stop=(k == 3))
        outt = apool.tile([128, CA], fp32, tag="outt")
        for n, (pso, psa) in enumerate([(ps_o0, ps_a20), (ps_o1, ps_a21)]):
            sl = slice(512 * n, 512 * (n + 1))
            tg = spool.tile([128, 512], fp32, tag="fin")
            nc.vector.tensor_add(tg[:], pso[:], outb_b[:, sl])
            nc.scalar.activation(tg[:], tg[:], AF.Sigmoid)
            nc.vector.tensor_mul(outt[:, sl], tg[:], psa[:])
        nc.sync.dma_start(out_p[:], outt[:])

    nc.compile()
    return nc


def _host_inputs(inputs):
    a = np.asarray(inputs["a"])[0]
    z = np.asarray(inputs["z"])[0]
    s = np.asarray(inputs["s"])[0]
    g = lambda k: np.asarray(inputs[k], np.float32)
    pb_wT = np.ascontiguousarray(g("pb_w").T).astype(BF16)
    pn_wT = np.ascontiguousarray(g("pn_w").T).astype(BF16)
    q_wT = np.ascontiguousarray(g("q_w").T).astype(BF16)
    kvg_wT = np.ascontiguousarray(g("kvg_w").T)
    perm = np.empty(3072, np.int64)
    for j in range(16):
        for v in range(3):
            perm[v * 1024 + j * 64:v * 1024 + j * 64 + 64] = np.arange(
                192 * j + 64 * v, 192 * j + 64 * v + 64)
    kvg_wT_p = np.ascontiguousarray(kvg_wT[:, perm]).astype(BF16)
    attn_wT = np.ascontiguousarray(g("attn_w").T).astype(BF16)
    out_wT = np.ascontiguousarray(g("out_w").T).astype(BF16)
    bias_wT = np.ascontiguousarray(g("bias_w").T)
    bias_wT2 = np.ascontiguousarray(np.concatenate([bias_wT, bias_wT], 0))
    pnw = g("pnorm_w").reshape(64, 1)
    pnw2 = np.ascontiguousarray(np.concatenate([pnw, pnw], 0))
    shared = dict(
        pb_wT=pb_wT, pn_wT=pn_wT, q_wT=q_wT, kvg_wT=kvg_wT_p,
        attn_wT=attn_wT, out_wT=out_wT, bias_wT2=bias_wT2, pnw2=pnw2,
        pnormb_col=np.ascontiguousarray(g("pnorm_b").reshape(64, 1)),
        biasb_col=np.ascontiguousarray(g("bias_b").reshape(16, 1)),
        snw4=np.ascontiguousarray(g("sn_w").reshape(4, 128).T),
        pb_b_r=np.ascontiguousarray(g("pb_b").reshape(1, CA)),
        qb_r=np.ascontiguousarray(g("q_b").reshape(1, CA)),
        outb_r=np.ascontiguousarray(g("out_b").reshape(1, CA)),
        id128=np.eye(128, dtype=np.float32).astype(BF16),
    )
    in_maps = []
    for m in range(NCORES):
        R = slice(128 * m, 128 * (m + 1))
        z_loc = z[R]                                       # [128, 1024, 64]
        zt = z_loc.transpose(0, 2, 1).reshape(64, 2, 64, S)  # [pair, par, cz, s2]
        im = dict(shared)
        im.update(
            a_loc=np.ascontiguousarray(a[R], dtype=np.float32),
            s_loc=np.ascontiguousarray(s[R], dtype=np.float32),
            sT_loc=np.ascontiguousarray(s[R].T).astype(BF16),
            z_t=np.ascontiguousarray(zt.reshape(64, 128, S)).astype(BF16),
        )
        in_maps.append(im)
    return in_maps


def kernel(**inputs):
    from concourse.bass_utils import run_bass_kernel_spmd
    key = "prog_dbg" if os.environ.get("KDEBUG") else "prog"
    if key not in _cache:
        _cache[key] = _build_program(debug=bool(os.environ.get("KDEBUG")))
    nc = _cache[key]
    in_maps = _host_inputs(inputs)
    res = run_bass_kernel_spmd(nc, in_maps, list(range(NCORES)),
                               trace=bool(os.environ.get("KTRACE")))
    kernel._last = res
    outs = [np.asarray(res.results[i]["out"], np.float32) for i in range(NCORES)]
    return np.concatenate(outs, 0)[None]
